# revision 1
# baseline (speedup 1.0000x reference)
"""GAT autoencoder kernel for trn2, 8-core SPMD — slot-aligned edge phase.

Design (v2):
  - Nodes sharded across 8 cores (6250/core, padded to 6656 = 13*512).
  - Per-core nodes are DEGREE-SORTED (by local in-degree incl self-loop);
    host permutes X columns accordingly (MSE is order-invariant).
  - Dense phases in feature-major orientation (features on partitions).
  - Per-layer node table rows [128 bf16]: [h(64) | a_s | a_d | zeros(62)],
    built locally, AllGathered to a global table GT in DRAM.
  - Edge phase: dst windows of 128 consecutive ranks; edges of the node at
    rank 128w+p occupy slots (p, s) for s < deg. One dma_gather per
    sub-chunk fetches 512B PAIR rows (2 nodes per row -> pair index fits
    int16 without group splits); a per-slot parity bit (host static)
    selects the half via weight splitting (w_lo/w_hi), so segment-softmax
    aggregation is one big multiply + one strided reduce per sub-chunk.
  - a_d per edge == a_d of the partition's node: a per-partition scalar
    captured during table build (advt column per window). No a_d gather,
    no scatter, no DRAM accumulator, no edge mask (pad slots point at a
    junk pair-row whose a_s is patched to -80 => exp weight ~ 0).
  - Final dense + MSE partial sum per core; host sums and divides.
"""

import numpy as np
import ml_dtypes

import bass_rust
import concourse.bass as bass
import concourse.bacc as bacc
import concourse.tile as tile
import concourse.mybir as mybir

BF16 = mybir.dt.bfloat16
F32 = mybir.dt.float32
FP8 = mybir.dt.float8e4
I16 = mybir.dt.int16
AF = mybir.ActivationFunctionType
ALU = mybir.AluOpType
bfdt = ml_dtypes.bfloat16


class Cfg:
    def __init__(self, N=50000, IN=1024, HD=256, ZD=64, NC=8, E=1600000,
                 MM=512, CAP=40, GCH=8, SCRATCH=16384, NQ=4):
        assert N % NC == 0
        self.N, self.IN, self.HD, self.ZD, self.NC, self.E = N, IN, HD, ZD, NC, E
        self.NL = N // NC                       # nodes per core
        self.MM = MM                            # dense m-chunk (<=512)
        self.NLP = ((self.NL + MM - 1) // MM) * MM   # padded local nodes
        assert self.NLP % 128 == 0
        self.CAP = CAP                          # max compute sub-chunk (slots/part)
        self.GCH = GCH                          # slot-cols per dma_gather (<=8)
        self.SCRATCH = SCRATCH                  # dynamic dma scratch bytes
        self.NQ = NQ                            # swdge queues
        self.EL = 64                            # row elements (bf16) = 128B:
                                                # [h fp8 x64 | a_s | a_d | pad]
        self.GROWS = self.NLP * NC              # global table rows
        assert self.GROWS // 2 - 1 <= 32767
        self.KT1 = IN // 128                    # k tiles for W1
        self.MH1 = HD // 128                    # m tiles for z1
        self.NMM = self.NLP // MM               # dense chunks
        self.TPC = MM // 128                    # transposes per chunk
        self.NWIN = (self.NL + 127) // 128      # real dst windows
        # per-window slot depth (max over cores); set by preprocess
        self.SW = None
        self.SLOTS = None


def preprocess(inputs, cfg):
    """Numpy sharding: returns in_maps (list of per-core input dicts)."""
    c = cfg
    X = np.asarray(inputs["X"])
    ei = np.asarray(inputs["edge_index"])
    src = np.concatenate([ei[0], np.arange(c.N, dtype=ei.dtype)]).astype(np.int64)
    dst = np.concatenate([ei[1], np.arange(c.N, dtype=ei.dtype)]).astype(np.int64)

    owner = dst // c.NL
    # degree-sorted rank per core + global table position of every node
    pos_global = np.zeros(c.N, np.int64)    # node -> owner*NLP + rank
    per_core = []
    for cc in range(c.NC):
        m = owner == cc
        d_loc = dst[m] - cc * c.NL
        deg = np.bincount(d_loc, minlength=c.NL)
        order = np.argsort(deg, kind="stable")      # rank -> local node
        rank = np.empty(c.NL, np.int64)             # local node -> rank
        rank[order] = np.arange(c.NL)
        pos_global[cc * c.NL:(cc + 1) * c.NL] = cc * c.NLP + rank
        per_core.append((m, d_loc, deg, order, rank))

    # per-window slot depth = max over cores of max degree in window
    SW = np.zeros(c.NWIN, np.int64)
    for cc in range(c.NC):
        _, _, deg, order, _ = per_core[cc]
        degs = deg[order]
        for w in range(c.NWIN):
            dw = degs[128 * w:128 * (w + 1)]
            if len(dw):
                SW[w] = max(SW[w], int(dw.max()))
    SW = np.maximum(SW, 1)
    c.SW = [int(s) for s in SW]
    woff = np.concatenate([[0], np.cumsum(128 * SW)])  # slot offset per window
    c.SLOTS = int(woff[-1])
    assert c.SLOTS % 128 == 0

    def wrap16(idx16):
        n = len(idx16)
        assert n % 16 == 0
        return np.ascontiguousarray(idx16.reshape(n // 16, 16).T)

    in_maps = []
    for cc in range(c.NC):
        m, d_loc, deg, order, rank = per_core[cc]
        s_e = src[m]
        junk_row = cc * c.NLP + c.NL            # even; patched a_s=-80
        gpair = np.full(c.SLOTS, junk_row // 2, np.int64)
        gpar = np.zeros(c.SLOTS, np.float32)
        # slot for edge e: rank r=rank[d_loc], w=r//128, p=r%128,
        # s = within-dst counter -> woff[w] + s*128 + p
        r_e = rank[d_loc]
        o_e = np.argsort(r_e, kind="stable")    # group edges by rank
        r_s = r_e[o_e]
        starts = np.searchsorted(r_s, np.arange(c.NL))
        s_within = np.arange(len(r_s)) - starts[r_s]
        slot = woff[r_s // 128] + s_within * 128 + (r_s % 128)
        g = pos_global[s_e[o_e]]
        gpair[slot] = g >> 1
        gpar[slot] = (g & 1).astype(np.float32)
        assert gpair.max() <= 32767 and gpair.min() >= 0

        Xc = X[cc * c.NL:(cc + 1) * c.NL][order]     # rank-permuted rows
        XT = np.zeros((c.IN, c.NLP), np.float32)
        XT[:, :c.NL] = Xc.T

        # interleaved parity pair (1-p, p) per slot: [128, SLOTS/128 * 2]
        p2d = gpar.reshape(c.SLOTS // 128, 128).T          # [128, S_tot]
        gpar2 = np.stack([1.0 - p2d, p2d], axis=2).reshape(128, -1)
        im = dict(
            XTb=XT.astype(bfdt),
            gidx=wrap16(gpair.astype(np.int16)),
            gpar2=np.ascontiguousarray(gpar2).astype(bfdt),
        )
        W = {k: np.asarray(v) for k, v in inputs.items()}
        im["W1b"] = W["W1"].astype(bfdt)
        im["W2b"] = W["W2"].astype(bfdt)
        im["g1Wb"] = W["g1W"].astype(bfdt)
        im["g2Wb"] = W["g2W"].astype(bfdt)
        im["g1a2"] = np.stack([W["g1as"], W["g1ad"]], 1).astype(bfdt)  # [ZD,2]
        im["g2a2"] = np.stack([W["g2as"], W["g2ad"]], 1).astype(bfdt)
        im["Wlb"] = W["Wl"].astype(bfdt)
        im["Wgb"] = W["Wg"].astype(bfdt)
        # loss-tail algebra: G = (Wd X)ᵀ-style needs Wdᵀ tiles; |X̂|² needs
        # M = Wd Wdᵀ; bias terms need u2 = 2 Wd bd and |bd|²
        im["WdTb"] = np.ascontiguousarray(
            W["Wd"].T.reshape(c.IN // 128, 128, c.ZD).transpose(1, 0, 2)
        ).astype(bfdt)                                     # [128, IN/128, ZD]
        im["Mtb"] = (W["Wd"] @ W["Wd"].T).astype(bfdt)     # [ZD, ZD]
        im["u2c"] = (2.0 * (W["Wd"] @ W["bd"])).reshape(c.ZD, 1).astype(np.float32)
        c.bd2 = float(np.sum(W["bd"].astype(np.float64) ** 2))
        im["b1c"] = np.ascontiguousarray(W["b1"].reshape(c.MH1, 128).T)  # [128, MH1]
        im["b2c"] = np.ascontiguousarray(W["b2"].reshape(c.MH1, 128).T)
        im["g1bc"] = W["g1b"].reshape(c.ZD, 1).astype(np.float32)
        im["g2bc"] = W["g2b"].reshape(c.ZD, 1).astype(np.float32)
        im["blc"] = W["bl"].reshape(c.ZD, 1).astype(np.float32)
        im["bgc"] = W["bg"].reshape(c.ZD, 1).astype(np.float32)
        im["bdc"] = np.ascontiguousarray(W["bd"].reshape(c.IN // 128, 128).T)  # [128, 8]
        in_maps.append(im)
    return in_maps


def build(cfg):
    c = cfg
    assert c.SW is not None, "preprocess first (sets SW/SLOTS)"
    nc = bacc.Bacc("TRN2", target_bir_lowering=False, debug=False,
                   num_devices=c.NC, dynamic_dma_scratch_size=c.SCRATCH,
                   num_swdge_queues=c.NQ)

    # ---- I/O ----
    XTb = nc.dram_tensor("XTb", [c.IN, c.NLP], BF16, kind="ExternalInput")
    gidx_d = nc.dram_tensor("gidx", [16, c.SLOTS // 16], I16, kind="ExternalInput")
    gpar2_d = nc.dram_tensor("gpar2", [128, (c.SLOTS // 128) * 2], BF16,
                             kind="ExternalInput")
    W1b = nc.dram_tensor("W1b", [c.IN, c.HD], BF16, kind="ExternalInput")
    W2b = nc.dram_tensor("W2b", [c.HD, c.HD], BF16, kind="ExternalInput")
    g1Wb = nc.dram_tensor("g1Wb", [c.HD, c.ZD], BF16, kind="ExternalInput")
    g2Wb = nc.dram_tensor("g2Wb", [c.ZD, c.ZD], BF16, kind="ExternalInput")
    g1a2 = nc.dram_tensor("g1a2", [c.ZD, 2], BF16, kind="ExternalInput")
    g2a2 = nc.dram_tensor("g2a2", [c.ZD, 2], BF16, kind="ExternalInput")
    Wlb = nc.dram_tensor("Wlb", [c.ZD, c.ZD], BF16, kind="ExternalInput")
    Wgb = nc.dram_tensor("Wgb", [c.ZD, c.ZD], BF16, kind="ExternalInput")
    WdTb = nc.dram_tensor("WdTb", [128, c.IN // 128, c.ZD], BF16,
                          kind="ExternalInput")
    Mtb = nc.dram_tensor("Mtb", [c.ZD, c.ZD], BF16, kind="ExternalInput")
    u2c = nc.dram_tensor("u2c", [c.ZD, 1], F32, kind="ExternalInput")
    b1c = nc.dram_tensor("b1c", [128, c.MH1], F32, kind="ExternalInput")
    b2c = nc.dram_tensor("b2c", [128, c.MH1], F32, kind="ExternalInput")
    g1bc = nc.dram_tensor("g1bc", [c.ZD, 1], F32, kind="ExternalInput")
    g2bc = nc.dram_tensor("g2bc", [c.ZD, 1], F32, kind="ExternalInput")
    blc = nc.dram_tensor("blc", [c.ZD, 1], F32, kind="ExternalInput")
    bgc = nc.dram_tensor("bgc", [c.ZD, 1], F32, kind="ExternalInput")
    bdc = nc.dram_tensor("bdc", [128, c.IN // 128], F32, kind="ExternalInput")
    loss_out = nc.dram_tensor("loss", [1, 1], F32, kind="ExternalOutput")

    # internal DRAM
    LT1 = nc.dram_tensor("LT1", [c.NLP, c.EL], BF16)
    LT2 = nc.dram_tensor("LT2", [c.NLP, c.EL], BF16)
    GT1 = nc.dram_tensor("GT1", [c.GROWS, c.EL], BF16, addr_space="Shared")
    GT2 = nc.dram_tensor("GT2", [c.GROWS, c.EL], BF16, addr_space="Shared")

    with tile.TileContext(nc) as tc:
        with (
            tc.tile_pool(name="const", bufs=1) as cpool,
            tc.tile_pool(name="xt", bufs=2) as xpool,
            tc.tile_pool(name="mm", bufs=3) as mpool,
            tc.tile_pool(name="ps", bufs=2, space="PSUM") as pspool,
            tc.tile_pool(name="psh", bufs=2, space="PSUM") as pshpool,
            tc.tile_pool(name="psa", bufs=2, space="PSUM") as psapool,
            tc.tile_pool(name="edge", bufs=4) as epool,
            tc.tile_pool(name="em", bufs=3) as empool,
            tc.tile_pool(name="zed", bufs=3) as zpool,
            tc.tile_pool(name="zf", bufs=2) as zfpool,
            tc.tile_pool(name="sq", bufs=1) as sqpool,
        ):
            # ---------- constants ----------
            w1t = cpool.tile([128, c.KT1, c.HD], BF16, tag="w1")
            nc.sync.dma_start(w1t[:], W1b.ap().rearrange("(a p) n -> p a n", p=128))
            w2t = cpool.tile([128, c.HD // 128, c.HD], BF16, tag="w2")
            nc.sync.dma_start(w2t[:], W2b.ap().rearrange("(a p) n -> p a n", p=128))
            g1wt = cpool.tile([128, c.HD // 128, c.ZD], BF16, tag="g1w")
            nc.sync.dma_start(g1wt[:], g1Wb.ap().rearrange("(a p) n -> p a n", p=128))
            g2wt = cpool.tile([c.ZD, c.ZD], BF16, tag="g2w")
            nc.sync.dma_start(g2wt[:], g2Wb.ap())
            g1at = cpool.tile([c.ZD, 2], BF16, tag="g1a")
            nc.sync.dma_start(g1at[:], g1a2.ap())
            g2at = cpool.tile([c.ZD, 2], BF16, tag="g2a")
            nc.sync.dma_start(g2at[:], g2a2.ap())
            wlt = cpool.tile([c.ZD, c.ZD], BF16, tag="wl")
            nc.sync.dma_start(wlt[:], Wlb.ap())
            wgt = cpool.tile([c.ZD, c.ZD], BF16, tag="wg")
            nc.sync.dma_start(wgt[:], Wgb.ap())
            wdTt = cpool.tile([128, c.IN // 128, c.ZD], BF16, tag="wdT")
            nc.sync.dma_start(wdTt[:], WdTb.ap())
            mtt = cpool.tile([c.ZD, c.ZD], BF16, tag="mt")
            nc.sync.dma_start(mtt[:], Mtb.ap())
            u2t = cpool.tile([c.ZD, 1], F32, tag="u2")
            nc.sync.dma_start(u2t[:], u2c.ap())
            b1t = cpool.tile([128, c.MH1], F32, tag="b1")
            nc.sync.dma_start(b1t[:], b1c.ap())
            b2t = cpool.tile([128, c.MH1], F32, tag="b2")
            nc.sync.dma_start(b2t[:], b2c.ap())
            g1bt = cpool.tile([c.ZD, 1], F32, tag="g1b")
            nc.sync.dma_start(g1bt[:], g1bc.ap())
            g2bt = cpool.tile([c.ZD, 1], F32, tag="g2b")
            nc.sync.dma_start(g2bt[:], g2bc.ap())
            blt = cpool.tile([c.ZD, 1], F32, tag="bl")
            nc.sync.dma_start(blt[:], blc.ap())
            bgt = cpool.tile([c.ZD, 1], F32, tag="bg")
            nc.sync.dma_start(bgt[:], bgc.ap())
            bdt = cpool.tile([128, c.IN // 128], F32, tag="bd")
            nc.sync.dma_start(bdt[:], bdc.ap())

            # edge metadata, SBUF-resident (idx tile [128, n/16]: 16-row wrap
            # replicated to the 8 gpsimd groups)
            gidx_t = cpool.tile([128, c.SLOTS // 16], I16, tag="gidx")
            for g in range(8):
                nc.sync.dma_start(gidx_t[:][16 * g:16 * (g + 1), :], gidx_d[:, :])
            gpar2_t = cpool.tile([128, (c.SLOTS // 128) * 2], BF16, tag="gpar2")
            nc.sync.dma_start(gpar2_t[:], gpar2_d[:])

            # a_d per rank-window, captured during table builds
            adv1 = cpool.tile([128, c.NLP // 128], F32, tag="adv1")
            adv2 = cpool.tile([128, c.NLP // 128], F32, tag="adv2")
            adv1s = cpool.tile([128, c.NLP // 128], F32, tag="adv1s")
            adv2s = cpool.tile([128, c.NLP // 128], F32, tag="adv2s")

            # loss-tail residents: G = Wdᵀ-contraction of X, per-chunk X sums
            Gt = cpool.tile([c.ZD, c.NMM, c.MM], F32, tag="Gt")
            SXt = cpool.tile([128, c.NMM, c.IN // 128], F32, tag="SXt")
            parts = cpool.tile([128, c.NMM], F32, tag="parts")
            parts2 = cpool.tile([c.ZD, c.NMM], F32, tag="parts2")

            # ---------- helper: table build tail (h -> TR -> LT) ----------
            def table_tail(hps, gat_w_a, LT, mi, advt):
                """hps: psum [ZD, MM] h-values. Builds TR rows, DMAs to LT,
                captures a_d column per 128-node block into advt."""
                TR = mpool.tile([128, c.MM], BF16, tag="tr")
                nc.scalar.activation(TR[:][0:c.ZD, :], hps[:], AF.Copy)
                aps = psapool.tile([2, c.MM], F32, tag="aps")
                nc.tensor.matmul(aps[:], gat_w_a[:], TR[:][0:c.ZD, :],
                                 start=True, stop=True)
                nc.vector.memset(TR[:][c.ZD:, :], 0)
                nc.scalar.activation(TR[:][c.ZD:c.ZD + 2, :], aps[:], AF.Copy)
                if mi * c.MM <= c.NL < (mi + 1) * c.MM:
                    # junk pair-row (NL, NL+1): a_s := -80 so pad-slot
                    # exp weights vanish
                    p0 = c.NL - mi * c.MM
                    nc.vector.memset(TR[:][c.ZD:c.ZD + 1, p0:p0 + 2], -80.0)
                for j in range(c.TPC):
                    w = mi * c.TPC + j
                    tro = mpool.tile([128, 128], BF16, tag="tro")
                    nc.sync.dma_start_transpose(
                        tro[:], TR[:][:, j * 128:(j + 1) * 128])
                    nc.vector.tensor_copy(advt[:][:, w:w + 1], tro[:][:, 65:66])
                    # pack 128B row: h as fp8 in bf16-elems 0:32, a_s/a_d at 32:34
                    row = mpool.tile([128, c.EL], BF16, tag="row")
                    nc.scalar.activation(row[:][:, 0:32].bitcast(FP8),
                                         tro[:][:, 0:c.ZD], AF.Copy)
                    nc.vector.tensor_copy(row[:][:, 32:34], tro[:][:, 64:66])
                    nc.sync.dma_start(
                        LT.ap()[mi * c.MM + j * 128: mi * c.MM + (j + 1) * 128, :],
                        row[:])

            # ---------- phase A: encoder MLP + table 1 + loss-side X stats --
            sqt = sqpool.tile([128, c.KT1, c.MM], F32, tag="sq")
            nc.vector.memset(parts[:], 0)
            for mi in range(c.NMM):
                sl = slice(mi * c.MM, (mi + 1) * c.MM)
                xk = xpool.tile([128, c.KT1, c.MM], BF16, tag="xk")
                nc.sync.dma_start(
                    xk[:], XTb.ap().rearrange("(a p) n -> p a n", p=128)[:, :, sl])
                # sum X^2 and sum X per chunk (loss terms), G = Wd-contraction
                nc.scalar.activation(sqt[:], xk[:], AF.Square,
                                     accum_out=parts[:][:, mi:mi + 1])
                nc.vector.tensor_reduce(SXt[:][:, mi, :], xk[:],
                                        mybir.AxisListType.X, ALU.add)
                gps = pshpool.tile([c.ZD, c.MM], F32, tag="hps")
                for k in range(c.KT1):
                    nc.tensor.matmul(gps[:], wdTt[:][:, k, :], xk[:][:, k, :],
                                     start=(k == 0), stop=(k == c.KT1 - 1))
                nc.scalar.activation(Gt[:][:, mi, :], gps[:], AF.Copy)
                z1 = mpool.tile([128, c.MH1, c.MM], BF16, tag="z1")
                for mh in range(c.MH1):
                    ps = pspool.tile([128, c.MM], F32, tag="ps")
                    for k in range(c.KT1):
                        nc.tensor.matmul(
                            ps[:], w1t[:][:, k, mh * 128:(mh + 1) * 128],
                            xk[:][:, k, :],
                            start=(k == 0), stop=(k == c.KT1 - 1))
                    nc.scalar.activation(z1[:][:, mh, :], ps[:],
                                         AF.Gelu, bias=b1t[:][:, mh:mh + 1])
                z2 = mpool.tile([128, c.MH1, c.MM], BF16, tag="z2")
                for mh in range(c.MH1):
                    ps = pspool.tile([128, c.MM], F32, tag="ps")
                    for k in range(c.HD // 128):
                        nc.tensor.matmul(
                            ps[:], w2t[:][:, k, mh * 128:(mh + 1) * 128],
                            z1[:][:, k, :],
                            start=(k == 0), stop=(k == c.HD // 128 - 1))
                    nc.scalar.activation(z2[:][:, mh, :], ps[:],
                                         AF.Gelu, bias=b2t[:][:, mh:mh + 1])
                hps = pshpool.tile([c.ZD, c.MM], F32, tag="hps")
                for k in range(c.HD // 128):
                    nc.tensor.matmul(hps[:], g1wt[:][:, k, :],
                                     z2[:][:, k, :],
                                     start=(k == 0), stop=(k == c.HD // 128 - 1))
                table_tail(hps, g1at, LT1, mi, adv1)

            # ---------- allgather table 1 ----------
            nc.gpsimd.collective_compute(
                "AllGather", ALU.bypass,
                replica_groups=[list(range(c.NC))],
                ins=[LT1.ap()], outs=[GT1.ap()])

            # ---------- edge phase: windowed gather + weighted reduce ----
            qn = [0]

            def edge_phase(GT, advt, advts, consumer):
                """consumer(mi, zfm): zfm [ZD, MM] bf16 normalized z chunk
                (pre-gelu, pre-bias)."""
                # adv*0.2 for the exp-of-leaky max trick:
                # exp(leaky(a_s+a_d)) = max(exp(a_s+a_d), exp(0.2(a_s+a_d)))
                nc.vector.tensor_scalar(out=advts[:], in0=advt[:], scalar1=0.2,
                                        scalar2=None, op0=ALU.mult)
                GTp = GT.ap().rearrange("(n two) e -> n (two e)", two=2)
                woff = 0
                zfm = None
                for w in range(c.NWIN):
                    S = c.SW[w]
                    mi, jw = w // c.TPC, w % c.TPC
                    if jw == 0:
                        zfm = zfpool.tile([128, c.MM], BF16, tag="zfm")
                        if mi == c.NMM - 1:
                            nc.vector.memset(zfm[:], 0)
                    red = zpool.tile([128, c.ZD], F32, tag="red")
                    wsum = zpool.tile([128, 1], F32, tag="wsum")
                    for ci, s0 in enumerate(range(0, S, c.CAP)):
                        sc = min(c.CAP, S - s0)
                        n = 128 * sc
                        off = woff + 128 * s0
                        hg = epool.tile([128, c.CAP, 2 * c.EL], BF16, tag="hg")
                        # >1024-descriptor gathers crash the SWDGE ring:
                        # split into <=GCH-column calls on rotating queues
                        for g0 in range(0, sc, c.GCH):
                            gsc = min(c.GCH, sc - g0)
                            og, ng = off + 128 * g0, 128 * gsc
                            nc.gpsimd.dma_gather(
                                out_ap=hg[:][:, g0:g0 + gsc, :],
                                in_ap=GTp,
                                idxs_ap=gidx_t[:][:, og // 16:(og + ng) // 16],
                                num_idxs=ng, num_idxs_reg=ng,
                                elem_size=2 * c.EL,
                                queue_num=qn[0] % c.NQ)
                            qn[0] += 1
                        # w = max(exp(a+adv), exp(0.2a+0.2adv)) for both
                        # halves, then split by parity pair (1-p, p)
                        aview = hg[:][:, 0:sc, :].rearrange(
                            "p s (two e) -> p s two e", two=2)[:, :, :, 32:33]
                        e1 = empool.tile([128, c.CAP, 2], F32, tag="e1")
                        e1s = e1[:][:, 0:sc, :].unsqueeze(3)
                        e2 = empool.tile([128, c.CAP, 2], F32, tag="e2")
                        e2s = e2[:][:, 0:sc, :].unsqueeze(3)
                        nc.scalar.activation(e1s, aview, AF.Exp,
                                             bias=advt[:][:, w:w + 1])
                        nc.scalar.activation(e2s, aview, AF.Exp,
                                             bias=advts[:][:, w:w + 1],
                                             scale=0.2)
                        nc.vector.tensor_tensor(out=e1s, in0=e1s, in1=e2s,
                                                op=ALU.max)
                        wb = empool.tile([128, c.CAP, 2], F32, tag="wb")
                        p2 = gpar2_t[:][:, 2 * (off // 128):
                                        2 * ((off + n) // 128)].rearrange(
                            "p (s two) -> p s two", two=2)
                        nc.vector.tensor_tensor(out=wb[:][:, 0:sc, :],
                                                in0=e1[:][:, 0:sc, :],
                                                in1=p2, op=ALU.mult)
                        # payload = h(lo/hi) * w(lo/hi); h arrives fp8-packed
                        hview = hg[:][:, 0:sc, :].rearrange(
                            "p s (two e) -> p s two e",
                            two=2)[:, :, :, 0:32].bitcast(FP8)
                        mpl = empool.tile([128, c.CAP, 2, c.ZD], BF16, tag="mpl")
                        nc.vector.tensor_tensor(
                            out=mpl[:][:, 0:sc, :, :], in0=hview,
                            in1=wb[:][:, 0:sc, :].unsqueeze(3).broadcast_to(
                                [128, sc, 2, c.ZD]),
                            op=ALU.mult)
                        # reduce over (s, parity) -> [128, ZD] / [128, 1]
                        mview = mpl[:][:, 0:sc, :, :].rearrange(
                            "p s two e -> p e s two")
                        wview = wb[:][:, 0:sc, :]
                        if ci == 0:
                            nc.vector.tensor_reduce(
                                red[:].unsqueeze(2).unsqueeze(3), mview,
                                mybir.AxisListType.XY, ALU.add)
                            nc.vector.tensor_reduce(
                                wsum[:].unsqueeze(2), wview,
                                mybir.AxisListType.XY, ALU.add)
                        else:
                            rt = zpool.tile([128, c.ZD], F32, tag="rt")
                            nc.vector.tensor_reduce(
                                rt[:].unsqueeze(2).unsqueeze(3), mview,
                                mybir.AxisListType.XY, ALU.add)
                            nc.vector.tensor_add(red[:], red[:], rt[:])
                            wt = zpool.tile([128, 1], F32, tag="wt")
                            nc.vector.tensor_reduce(
                                wt[:].unsqueeze(2), wview,
                                mybir.AxisListType.XY, ALU.add)
                            nc.vector.tensor_add(wsum[:], wsum[:], wt[:])
                    # normalize -> bf16 node-major -> transpose into zfm col
                    nc.vector.tensor_scalar_max(wsum[:], wsum[:], 1e-30)
                    nc.vector.reciprocal(wsum[:], wsum[:])
                    znm = zpool.tile([128, 128], BF16, tag="znm")
                    if w < 3:   # must cover every rotating buffer (bufs=3)
                        nc.vector.memset(znm[:][:, c.ZD:], 0)
                    nc.vector.tensor_scalar(
                        out=znm[:][:, 0:c.ZD], in0=red[:], scalar1=wsum[:],
                        scalar2=None, op0=ALU.mult)
                    nc.sync.dma_start_transpose(
                        zfm[:][:, jw * 128:(jw + 1) * 128], znm[:])
                    woff += 128 * S
                    if jw == c.TPC - 1 or w == c.NWIN - 1:
                        consumer(mi, zfm)

            # ---------- layer-1 consumer: gelu+bias, h2, table 2 ----------
            def build_table2(mi, zfm):
                zg = zfpool.tile([c.ZD, c.MM], BF16, tag="zg")
                nc.scalar.activation(zg[:], zfm[:][0:c.ZD, :], AF.Gelu,
                                     bias=g1bt[:])
                hps = pshpool.tile([c.ZD, c.MM], F32, tag="hps")
                nc.tensor.matmul(hps[:], g2wt[:], zg[:], start=True, stop=True)
                table_tail(hps, g2at, LT2, mi, adv2)

            edge_phase(GT1, adv1, adv1s, build_table2)

            nc.gpsimd.collective_compute(
                "AllGather", ALU.bypass,
                replica_groups=[list(range(c.NC))],
                ins=[LT2.ap()], outs=[GT2.ap()])

            # ---------- layer-2 consumer: tail via loss algebra ----------
            # |X̂-X|² = ΣX² - 2(z5·G + bd·SX) + (z5·(M z5) + u2·z5 + |bd|²nv)
            def loss_tail(mi, zfm):
                zg = zfpool.tile([c.ZD, c.MM], BF16, tag="zg")
                nc.scalar.activation(zg[:], zfm[:][0:c.ZD, :], AF.Gelu,
                                     bias=g2bt[:])
                z4 = zfpool.tile([c.ZD, c.MM], BF16, tag="z4")
                ps = pshpool.tile([c.ZD, c.MM], F32, tag="hps")
                nc.tensor.matmul(ps[:], wlt[:], zg[:], start=True, stop=True)
                nc.scalar.activation(z4[:], ps[:], AF.Identity, bias=blt[:])
                z5 = zfpool.tile([c.ZD, c.MM], BF16, tag="z5")
                ps2 = pshpool.tile([c.ZD, c.MM], F32, tag="hps")
                nc.tensor.matmul(ps2[:], wgt[:], z4[:], start=True, stop=True)
                nc.scalar.activation(z5[:], ps2[:], AF.Identity, bias=bgt[:])
                nv = min(c.NL - mi * c.MM, c.MM)
                if nv <= 0:
                    return
                pm = pshpool.tile([c.ZD, c.MM], F32, tag="hps")
                nc.tensor.matmul(pm[:], mtt[:], z5[:], start=True, stop=True)
                tmp = zfpool.tile([c.ZD, c.MM], F32, tag="tmp")
                t = tmp[:][:, 0:nv]
                nc.vector.scalar_tensor_tensor(
                    out=t, in0=Gt[:][:, mi, 0:nv], scalar=-2.0,
                    in1=pm[:][:, 0:nv], op0=ALU.mult, op1=ALU.add)
                nc.vector.tensor_scalar(out=t, in0=t, scalar1=u2t[:],
                                        scalar2=None, op0=ALU.add)
                nc.vector.tensor_mul(t, t, z5[:][:, 0:nv])
                nc.vector.tensor_reduce(parts2[:][:, mi:mi + 1], t,
                                        mybir.AxisListType.X, ALU.add)

            edge_phase(GT2, adv2, adv2s, loss_tail)

            # combine partials:
            #   tot = Σ parts (ΣX²) - 2·Σ(SX·bd) + Σ parts2, over this core
            tot = cpool.tile([128, 1], F32, tag="tot")
            nc.vector.tensor_reduce(tot[:], parts[:], mybir.AxisListType.X,
                                    ALU.add)
            sxb = cpool.tile([128, c.NMM, c.IN // 128], F32, tag="sxb")
            nc.vector.tensor_tensor(
                out=sxb[:], in0=SXt[:],
                in1=bdt[:].unsqueeze(1).broadcast_to(
                    [128, c.NMM, c.IN // 128]),
                op=ALU.mult)
            sxr = cpool.tile([128, 1], F32, tag="sxr")
            nc.vector.tensor_reduce(sxr[:].unsqueeze(2), sxb[:],
                                    mybir.AxisListType.XY, ALU.add)
            nc.vector.scalar_tensor_tensor(
                out=tot[:], in0=sxr[:], scalar=-2.0, in1=tot[:],
                op0=ALU.mult, op1=ALU.add)
            t2 = cpool.tile([c.ZD, 1], F32, tag="t2")
            nc.vector.tensor_reduce(t2[:], parts2[:], mybir.AxisListType.X,
                                    ALU.add)
            nc.vector.tensor_add(tot[:][0:c.ZD, :], tot[:][0:c.ZD, :], t2[:])
            tot2 = cpool.tile([128, 1], F32, tag="tot2")
            nc.gpsimd.partition_all_reduce(tot2[:], tot[:], channels=128,
                                           reduce_op=bass_rust.ReduceOp.add)
            nc.sync.dma_start(loss_out.ap(), tot2[:][0:1, 0:1])

    nc.compile()
    return nc


def postprocess(results, cfg):
    tot = sum(float(r["loss"][0, 0]) for r in results)
    tot += cfg.N * getattr(cfg, "bd2", 0.0)   # |bd|^2 per node
    return np.array(tot / (cfg.N * cfg.IN), dtype=np.float32)


# ---------------------------------------------------------------------------
# public entry point
# ---------------------------------------------------------------------------
_CACHE = {}


def _get_program(cfg):
    key = (tuple(cfg.SW), cfg.CAP, cfg.SCRATCH, cfg.NQ)
    if key not in _CACHE:
        _CACHE[key] = build(cfg)
    return _CACHE[key]


def kernel(**inputs) -> np.ndarray:
    from concourse.bass_utils import run_bass_kernel_spmd
    cfg = Cfg()
    in_maps = preprocess(inputs, cfg)
    nc = _get_program(cfg)
    res = run_bass_kernel_spmd(nc, in_maps, list(range(cfg.NC)))
    return postprocess(res.results, cfg)

